# revision 1
# baseline (speedup 1.0000x reference)
"""Chamfer rate-distortion loss on 8 TRN2 NeuronCores.

Layout: 8 cores = 4 batches x 2 chamfer directions. Each core computes, for
its (batch, direction), the per-point nearest-neighbor squared distance of
8192 query points X against 8192 reference points Y.

Device algorithm per core:
  - X and Y are pre-sorted (host) along coordinate AXIS.
  - matmul trick (fp16 hi/lo split, K=13, ~1e-6 rel precision):
    PSUM[m,p] = SCALE^2*(|x_m|^2 - 2 x_m.y_p + |y_p|^2) = SCALE^2*D[m,p] >= 0.
    Including |x|^2 via two extra contraction rows keeps PSUM values small
    and non-negative, so fp16 intermediates in the reduction stay accurate
    (overflow to +inf is min-safe).
  - 64 chunks of 128 sorted queries each scan a 192-wide window of sorted Y
    centered on their own sorted position (guard G=32 each side); edges are
    padded with duplicates of the extreme real points (which can never lower
    a min below the true min).
  - The wr input is laid out in 4 column-shards at SBUF partition groups
    {0,32,64,96} (13 rows each) so the HBM load spreads over 52 partition
    lines instead of 13; six interleaved column-slice DMAs split over the SP
    and Scalar queues let the first matmuls start while the bulk streams in.
  - min-reduction is split across two engines: for 7 of 8 chunk-groups the
    ACT engine drains PSUM -> SBUF fp16 and DVE folds in fp16 (2x mode)
    then reduces; the last group is DVE sub-reduces straight from PSUM so
    the post-matmul tail is one small reduce.
    (GPSIMD cannot touch PSUM and its codegen has no min/max TensorTensor;
    DVE TensorTensor may read at most one PSUM operand; tensor_reduce has
    no 2x modes. These constraints shape the split.)

Exactness: for a query x, every Y outside its window differs from x along
the sort axis by at least gap(x), so any excluded point has D >= gap^2.
The host flags points whose Morton-candidate NN upper bound exceeds their
gap ("hard" points, data-dependent but sound) and recomputes them exactly
on the host; a post-hoc check dmin <= gap^2 - margin catches any residual
device noise and falls back to exact host recompute for those rows.
"""

import os

import numpy as np

B, M, P = 4, 8192, 8192
AXIS = 2
SUB = 128            # chunk: 128 sorted queries share one window
GUARD = 32           # guaranteed positions each side beyond the chunk span
BAND = SUB + 2 * GUARD   # 192 window width per chunk
PAD = GUARD          # edge-dup pad; window for chunk u = rt cols [128u, 128u+224)
NBLK = M // SUB      # 64 chunks
GRP = 8              # chunks per PSUM tile / consumer group
NSHARD = 4           # column shards at partition groups 0/32/64/96
NCH_S = 16           # chunks per shard
WTS = M // NSHARD        # 2048 wt cols per shard
RTS = WTS + 2 * PAD      # 2144 rt cols per shard (96-col overlap)
COLS = WTS + RTS         # 4192 cols per shard line
KROWS = 13           # fp16 hi/lo decomposition rows (see _prep_core)
SCALE = 32.0         # coordinate pre-scale; device min is SCALE^2 * real
LMBDA = 5.0
PATHS = "bbbbbbbd"  # b=ACT drain + DVE fp16 folds, d=DVE sub-reduces (last: short tail)

_CACHE = {}
LAST_RESULTS = None


def _build_bass():
    import concourse.tile as tile
    from concourse import bacc, mybir

    nc = bacc.Bacc(None, target_bir_lowering=False, debug=False)
    f32 = mybir.dt.float32
    f16 = mybir.dt.float16
    bf16 = mybir.dt.bfloat16
    MIN = mybir.AluOpType.min
    X = mybir.AxisListType.X

    wr_d = nc.dram_tensor("wr", [128, COLS], f16, kind="ExternalInput")
    out_d = nc.dram_tensor("out", [128, NBLK], f32, kind="ExternalOutput")

    with tile.TileContext(nc) as tc:
        with (
            tc.tile_pool(name="const", bufs=1) as cpool,
            tc.tile_pool(name="outp", bufs=1) as opool,
            tc.tile_pool(name="sba", bufs=2) as apool,
            tc.tile_pool(name="psum", bufs=2, space="PSUM") as ppool,
        ):
            wr = cpool.tile([128, COLS], f16)
            # rt/wt slices interleaved across the SP and Scalar queues so
            # early chunks' inputs land first while the bulk streams behind
            WS, RS = WTS // 3, RTS // 3
            for k in range(3):
                nc.sync.dma_start(
                    wr[:, WTS + RS * k:WTS + (RS * (k + 1) if k < 2 else RTS)],
                    wr_d[:, WTS + RS * k:WTS + (RS * (k + 1) if k < 2 else RTS)])
                nc.scalar.dma_start(
                    wr[:, WS * k:WS * (k + 1) if k < 2 else WTS],
                    wr_d[:, WS * k:WS * (k + 1) if k < 2 else WTS])
            outt = opool.tile([128, NBLK], f32)

            H1, H2, H3 = BAND // 2, BAND // 4, BAND // 8  # 96/48/24

            def emit_mms(k):
                # chunk stride padded to 256 f32 so each 192-col matmul
                # output stays inside one 2KB PSUM bank
                ps = ppool.tile([128, GRP, 256], f32, tag="ps")
                for g in range(GRP):
                    u = GRP * k + g           # global chunk index
                    s = u // NCH_S            # shard / partition group
                    p0 = 32 * s
                    wc = SUB * u - WTS * s          # wt col within shard
                    rc = SUB * u - WTS * s + WTS    # rt col within shard line
                    nc.tensor.matmul(
                        ps[:, g, 0:BAND],
                        wr[p0:p0 + KROWS, wc:wc + SUB],
                        wr[p0:p0 + KROWS, rc:rc + BAND],
                        start=True, stop=True,
                        tile_position=(p0, 0),
                    )
                return ps

            def emit_consumers(k, ps):
                ob = outt[:, GRP * k:GRP * (k + 1)]
                if PATHS[k] == "b":
                    # ACT drains PSUM -> SBUF fp16 (values >= 0; overflow to
                    # +inf is min-safe); DVE folds in fp16 (2x mode)
                    sa = apool.tile([128, GRP, BAND], f16, tag="sa")
                    nc.scalar.copy(sa[:], ps[:, :, 0:BAND])
                    f1 = apool.tile([128, GRP, H1], f16, tag="f1")
                    f2 = apool.tile([128, GRP, H2], f16, tag="f2")
                    f3 = apool.tile([128, GRP, H3], f16, tag="f3")
                    nc.vector.tensor_tensor(f1[:], sa[:, :, 0:H1],
                                            sa[:, :, H1:BAND], op=MIN)
                    nc.vector.tensor_tensor(f2[:], f1[:, :, 0:H2],
                                            f1[:, :, H2:H1], op=MIN)
                    nc.vector.tensor_tensor(f3[:], f2[:, :, 0:H3],
                                            f2[:, :, H3:H2], op=MIN)
                    nc.vector.tensor_reduce(ob, f3[:], axis=X, op=MIN)
                else:
                    # DVE sub-reduces straight from PSUM: each starts as soon
                    # as its pair of matmuls lands (short tail)
                    for h in range(0, GRP, 2):
                        nc.vector.tensor_reduce(
                            outt[:, GRP * k + h:GRP * k + h + 2],
                            ps[:, h:h + 2, 0:BAND], axis=X, op=MIN)

            NG = NBLK // GRP
            for k in range(NG - 2):
                ps = emit_mms(k)
                emit_consumers(k, ps)
                if k == NG // 2 - 1:
                    nc.sync.dma_start(out_d[:, 0:NBLK // 2],
                                      outt[:, 0:NBLK // 2])
            # last two groups: emit group NG-1's matmuls and its PSUM
            # sub-reduces BEFORE group NG-2's consumer chain so the in-order
            # DVE queue drains the final PSUM tiles right behind the PE
            # instead of stalling behind the fold chain
            ps6 = emit_mms(NG - 2)
            ps7 = emit_mms(NG - 1)
            emit_consumers(NG - 1, ps7)
            emit_consumers(NG - 2, ps6)
            nc.sync.dma_start(out_d[:, NBLK // 2:], outt[:, NBLK // 2:])
    nc.compile()
    return nc


def _morton_key(pts):
    rng = pts.max(0) - pts.min(0)
    q = ((pts - pts.min(0)) / (rng + 1e-9) * 1023).astype(np.uint64)

    def spread(x):
        x = x & np.uint64(0x3FF)
        x = (x | (x << np.uint64(16))) & np.uint64(0x30000FF)
        x = (x | (x << np.uint64(8))) & np.uint64(0x300F00F)
        x = (x | (x << np.uint64(4))) & np.uint64(0x30C30C3)
        x = (x | (x << np.uint64(2))) & np.uint64(0x9249249)
        return x

    return (spread(q[:, 0]) | (spread(q[:, 1]) << np.uint64(1))
            | (spread(q[:, 2]) << np.uint64(2)))


def _prep_core(X, Y):
    """Host prep for one (batch, direction): returns in_map plus the metadata
    needed to verify and assemble the result."""
    xo = np.argsort(X[:, AXIS], kind="stable")
    yo = np.argsort(Y[:, AXIS], kind="stable")
    Xs = X[xo]
    Ys = Y[yo]
    X2 = (Xs.astype(np.float64) ** 2).sum(1)
    Y2 = (Ys.astype(np.float64) ** 2).sum(1)
    zx = Xs[:, AXIS].astype(np.float64)
    zy = Ys[:, AXIS].astype(np.float64)

    # gap to nearest excluded Y along the sort axis, per query
    i = np.arange(M)
    c = i // SUB
    lo_pos = SUB * c - GUARD         # first included Y position
    hi_pos = SUB * c + SUB + GUARD   # first excluded upper position
    gap = np.full(M, np.inf)
    has_lo = lo_pos > 0
    gap[has_lo] = zx[has_lo] - zy[lo_pos[has_lo] - 1]
    has_hi = hi_pos < P
    gap[has_hi] = np.minimum(gap[has_hi], zy[hi_pos[has_hi]] - zx[has_hi])
    gap = np.maximum(gap, 0.0)

    # conservative NN-distance upper bound via Morton-order neighbors
    allpts = np.concatenate([Xs, Ys]).astype(np.float64)
    mk = _morton_key(allpts)
    inv = np.empty(2 * M, dtype=np.int64)
    inv[np.argsort(mk, kind="stable")] = np.arange(2 * M)
    y_rank = inv[M:]
    order_y = np.argsort(y_rank, kind="stable")
    sorted_ranks = y_rank[order_y]
    K = 16
    idx = np.searchsorted(sorted_ranks, inv[:M])
    cand = np.clip(idx[:, None] + np.arange(-K, K)[None, :], 0, M - 1)
    cands = order_y[cand]
    d2 = ((Xs[:, None, :].astype(np.float64) - Ys[cands].astype(np.float64)) ** 2).sum(-1)
    d_cap2 = d2.min(1)

    hard = np.flatnonzero(~(d_cap2 <= (gap * gap) * 0.98))

    # fp16 hi/lo decomposition of SCALE*X and SCALE*Y; device computes
    # SCALE^2 * (|x|^2 - 2 x.y + |y|^2) in fp32 PSUM via K=13 rows:
    #   r0-2: -2*a_d * c_d     r3-5: -2*a_d * e_d     r6-8: -2*b_d * c_d
    #   r9:   1 * w_hi         r10:  1 * w_lo
    #   r11:  v_hi * 1         r12:  v_lo * 1
    # where a+b ~ SCALE*x, c+e ~ SCALE*y, w_hi+w_lo ~ |SCALE*y|^2,
    # v_hi+v_lo ~ |SCALE*x|^2.
    Xss = (SCALE * Xs).astype(np.float64)
    Yss = (SCALE * Ys).astype(np.float64)
    a = Xss.astype(np.float16)
    bb = (Xss - a.astype(np.float64)).astype(np.float16)
    cc = Yss.astype(np.float16)
    e = (Yss - cc.astype(np.float64)).astype(np.float16)
    w = (Yss ** 2).sum(1)
    wh = w.astype(np.float16)
    wl = (w - wh.astype(np.float64)).astype(np.float16)
    v = (Xss ** 2).sum(1)
    vh = v.astype(np.float16)
    vl = (v - vh.astype(np.float64)).astype(np.float16)

    na = (-2.0 * a.astype(np.float64)).astype(np.float16)  # exact: x2 of fp16
    nb = (-2.0 * bb.astype(np.float64)).astype(np.float16)

    wt = np.empty((KROWS, M), dtype=np.float16)
    wt[0:3, :] = na.T
    wt[3:6, :] = na.T
    wt[6:9, :] = nb.T
    wt[9:11, :] = 1.0
    wt[11, :] = vh
    wt[12, :] = vl

    rt = np.empty((KROWS, P + 2 * PAD), dtype=np.float16)
    ccT = cc.T
    eeT = e.T
    # edge-duplicate padding: repeats of the first/last sorted reference
    # point — real candidates, can never lower a min below the true min.
    for cols, sl in ((slice(0, PAD), 0), (slice(PAD + P, P + 2 * PAD), P - 1)):
        rt[0:3, cols] = ccT[:, sl:sl + 1]
        rt[3:6, cols] = eeT[:, sl:sl + 1]
        rt[6:9, cols] = ccT[:, sl:sl + 1]
        rt[9, cols] = wh[sl]
        rt[10, cols] = wl[sl]
    rt[0:3, PAD:PAD + P] = ccT
    rt[3:6, PAD:PAD + P] = eeT
    rt[6:9, PAD:PAD + P] = ccT
    rt[9, PAD:PAD + P] = wh
    rt[10, PAD:PAD + P] = wl
    rt[11:13, :] = 1.0

    # shard layout: partition group 32s holds wt cols [2048s, 2048(s+1)) and
    # rt cols [2048s, 2048s+2144) (rt in padded coords; windows for chunks
    # 16s..16s+15 fall inside because of the 96-col overlap)
    wr = np.zeros((128, COLS), dtype=np.float16)
    for s in range(NSHARD):
        wr[32 * s:32 * s + KROWS, 0:WTS] = wt[:, WTS * s:WTS * (s + 1)]
        wr[32 * s:32 * s + KROWS, WTS:COLS] = rt[:, WTS * s:WTS * s + RTS]

    return {"wr": wr}, {
        "Xs": Xs.astype(np.float64), "Ys": Ys.astype(np.float64),
        "X2": X2, "Y2": Y2, "gap": gap, "hard": hard,
    }


def _exact_rows(meta, idx):
    """Exact NN distance (float64) for query rows idx against all of Y."""
    Xb = meta["Xs"][idx]
    D = meta["X2"][idx][:, None] + meta["Y2"][None, :] - 2.0 * (Xb @ meta["Ys"].T)
    return D.min(axis=1)


def _post_core(out, meta):
    """Combine device output into sum over queries of min-D (float64)."""
    inv_s2 = 1.0 / (SCALE * SCALE)
    dmin = out.T.reshape(M).astype(np.float64) * inv_s2

    if len(meta["hard"]):
        dmin[meta["hard"]] = _exact_rows(meta, meta["hard"])

    # soundness check for window-only points: device numeric margin includes
    # the bf16 rounding of intermediates (rel ~8e-3) plus fp16 product noise
    g2 = meta["gap"] * meta["gap"]
    ok = dmin <= g2 - 2e-3 * inv_s2 - 8e-3 * np.abs(dmin)
    ok[meta["hard"]] = True
    bad = np.flatnonzero(~ok)
    if len(bad):
        dmin[bad] = _exact_rows(meta, bad)
    if os.environ.get("CHAMFER_DEBUG"):
        print(f"  host-recomputed: hard={len(meta['hard'])} bad={len(bad)}")
    return dmin.sum()


def _install_axon_profile_hook():
    """Make trace=True work under axon when the image's antenv lacks
    axon_hooks: inject a shim module wired to the ctypes NTFF driver."""
    import sys
    import types
    try:
        from antenv.axon_hooks import get_axon_ntff_profile_hook  # noqa: F401
        return
    except ImportError:
        pass
    try:
        import antenv
        from trn_agent_boot.trn_boot import _ntff_profile_via_ctypes
        hook = _ntff_profile_via_ctypes("/opt/axon/libaxon_pjrt.so")
    except Exception:
        hook = None
    mod = types.ModuleType("antenv.axon_hooks")
    state = {"h": hook}
    mod.get_axon_ntff_profile_hook = lambda: state["h"]
    mod.set_axon_ntff_profile_hook = lambda h: state.__setitem__("h", h)
    sys.modules["antenv.axon_hooks"] = mod
    try:
        antenv.axon_hooks = mod
    except Exception:
        pass


def kernel(x_hat, points, likelihoods):
    from concourse.bass_utils import run_bass_kernel_spmd
    global LAST_RESULTS

    trace = bool(int(os.environ.get("CHAMFER_TRACE", "0")))
    if trace:
        _install_axon_profile_hook()

    if "nc" not in _CACHE:
        _CACHE["nc"] = _build_bass()
    nc = _CACHE["nc"]

    in_maps, metas = [], []
    for core in range(8):
        b, d = core // 2, core % 2
        X = x_hat[b] if d == 0 else points[b]
        Y = points[b] if d == 0 else x_hat[b]
        m, meta = _prep_core(np.asarray(X), np.asarray(Y))
        in_maps.append(m)
        metas.append(meta)

    res = run_bass_kernel_spmd(
        nc, in_maps, core_ids=list(range(8)), trace=trace,
    )
    LAST_RESULTS = res

    sums = [_post_core(res.results[c]["out"], metas[c]) for c in range(8)]
    cham_x = sum(sums[c] for c in range(8) if c % 2 == 0) / (B * M)
    cham_y = sum(sums[c] for c in range(8) if c % 2 == 1) / (B * P)
    rec = cham_x + cham_y

    lik = np.asarray(likelihoods, dtype=np.float64)
    bpp = np.log2(lik).sum() / (-(B * P))

    loss = bpp + LMBDA * rec
    return np.array([loss, bpp, rec], dtype=np.float32)



# revision 7
# speedup vs baseline: 1.0472x; 1.0472x over previous
"""Chamfer rate-distortion loss on 8 TRN2 NeuronCores.

Layout: 8 cores = 4 batches x 2 chamfer directions. Each core computes, for
its (batch, direction), the per-point nearest-neighbor squared distance of
8192 query points X against 8192 reference points Y.

Device algorithm per core (v2):
  - X and Y are pre-sorted (host) along coordinate AXIS.
  - matmul trick (fp16 hi/lo split, K=13 rows/chunk, ~1e-6 rel precision):
    PSUM[m,p] = SCALE^2*(|x_m|^2 - 2 x_m.y_p + |y_p|^2) = SCALE^2*D[m,p] >= 0.
    Including |x|^2 keeps PSUM values small and non-negative so the fp16
    drain stays accurate (overflow to +inf is min-safe).
  - 64 chunks of 128 sorted queries each scan a 160-wide window of sorted Y
    (guard G=16 each side); edges padded with duplicates of the extreme
    real points (which can never lower a min below the true min).
  - K-packing: 2 chunks stacked along the contraction dim (26 rows) with
    block-diagonal moving data (the off-chunk halves are zero), so one
    LDWEIGHTS+MATMUL covers 2 chunks (N=320, one PSUM bank). Halves the
    weight-load traffic and instruction count.
  - The 32 packs rotate over PE row groups 0/32/64/96 (tile_position), so
    consecutive matmuls hit different row groups: the PE runs them
    concurrently and pulls next weights ahead of in-flight matmuls.
  - PSUM super-tiles [128, 4chunks, 256] (2 banks); consumers split across
    engines to balance: 3 of 4 super-tiles drain via ACT (PSUM->SBUF fp16)
    followed by one DVE tensor_tensor_reduce per chunk (fused pair-min +
    full min-reduce); the 4th reduces directly from PSUM fp32 with one DVE
    tensor_reduce. (ACT ~1 elem/cyc@1.2G; DVE 1 elem/cyc fp32, TTR cheap.)

Exactness: for a query x, every Y outside its window differs from x along
the sort axis by at least gap(x), so any excluded point has D >= gap^2.
The host flags points whose Morton-candidate NN upper bound exceeds their
gap ("hard" points, data-dependent but sound) and recomputes them exactly
on the host; a post-hoc check dmin <= gap^2 - margin catches any residual
device noise and falls back to exact host recompute for those rows.
"""

import os

import numpy as np

B, M, P = 4, 8192, 8192
AXIS = 2
SUB = 128            # chunk: 128 sorted queries share one window
GUARD = 16           # guaranteed positions each side beyond the chunk span
BAND = SUB + 2 * GUARD   # 160 window width per chunk
HALF = BAND // 2
PAD = GUARD          # edge-dup pad; window for chunk c = rt cols [128c, 128c+160)
NBLK = M // SUB      # 64 chunks
NPACK = NBLK // 2    # 32 two-chunk packs
NSUPER = NPACK // 2  # 16 PSUM super-tiles (4 chunks each)
NJ = NPACK // 4      # 8 local packs per row group
KROWS = 13           # fp16 hi/lo decomposition rows (see _prep_core)
K2 = 2 * KROWS       # 26 contraction rows per pack
WTC = NJ * SUB       # 1024 weight cols per row group line
RTC = NJ * BAND      # 1280 rt cols per block per row group line
SCALE = 32.0         # coordinate pre-scale; device min is SCALE^2 * real
LMBDA = 5.0
# consumer path per super-tile: a = ACT drain + DVE TTR, d = DVE reduce from PSUM
PATHS = "aadaaadaaadaaada"

_CACHE = {}
LAST_RESULTS = None


def _build_bass():
    import concourse.tile as tile
    from concourse import bacc, mybir

    nc = bacc.Bacc(None, target_bir_lowering=False, debug=False)
    f32 = mybir.dt.float32
    f16 = mybir.dt.float16
    MIN = mybir.AluOpType.min
    X = mybir.AxisListType.X
    COPY = mybir.ActivationFunctionType.Copy

    wt_d = nc.dram_tensor("wt", [128, WTC], f16, kind="ExternalInput")
    rt_d = nc.dram_tensor("rt", [128, 2, NJ, BAND], f16, kind="ExternalInput")
    out_d = nc.dram_tensor("out", [128, NBLK], f32, kind="ExternalOutput")

    with tile.TileContext(nc) as tc:
        with (
            tc.tile_pool(name="const", bufs=1) as cpool,
            tc.tile_pool(name="outp", bufs=1) as opool,
            tc.tile_pool(name="sba", bufs=3) as apool,
            tc.tile_pool(name="psum", bufs=3, space="PSUM") as ppool,
            tc.tile_pool(name="psumd", bufs=1, space="PSUM") as ppoold,
        ):
            wtile = cpool.tile([128, WTC], f16)
            rtile = cpool.tile([128, 2, NJ, BAND], f16)
            outt = opool.tile([128, NBLK], f32)

            # input DMA split over the SP and Scalar queues; early packs'
            # slices first so the first matmuls start while the bulk streams
            nc.sync.dma_start(wtile[:, 0:2 * SUB], wt_d[:, 0:2 * SUB])
            nc.scalar.dma_start(wtile[:, 2 * SUB:WTC], wt_d[:, 2 * SUB:WTC])
            nc.sync.dma_start(rtile[:, 0, 0:2, :], rt_d[:, 0, 0:2, :])
            nc.sync.dma_start(rtile[:, 1, 0:2, :], rt_d[:, 1, 0:2, :])
            nc.scalar.dma_start(rtile[:, 0, 2:NJ, :], rt_d[:, 0, 2:NJ, :])
            nc.sync.dma_start(rtile[:, 1, 2:NJ, :], rt_d[:, 1, 2:NJ, :])

            def pack_aps(s, h):
                p = 2 * s + h          # global pack index
                rg = p % 4             # PE row group (rotates)
                j = p // 4             # local pack within row group
                p0 = 32 * rg
                return (wtile[p0:p0 + K2, SUB * j:SUB * (j + 1)],
                        rtile[p0:p0 + K2, :, j, :], p0, j)

            for s in range(NSUPER):
                if PATHS[s] == "a":
                    # one matmul per pack: the pack's 26 K-rows are the
                    # stationary tile; the moving data is block-diagonal
                    # (each chunk's half is zero in the other's rows), so
                    # cols 0:160 are chunk a's D and 160:320 chunk b's
                    ps = ppool.tile([128, 2, 512], f32, tag="psa")
                    for h in range(2):
                        wa, ra, p0, j = pack_aps(s, h)
                        nc.tensor.matmul(
                            ps[:, h, 0:2 * BAND], wa, ra,
                            start=True, stop=True, tile_position=(p0, 0),
                        )
                    # ACT drains PSUM -> SBUF fp16 (values >= 0; overflow to
                    # +inf is min-safe); DVE folds halves at 2x then reduces
                    sa = apool.tile([128, 2, 2, BAND], f16, tag="sa")
                    nc.scalar.activation(sa[:], ps[:, :, 0:2 * BAND], COPY)
                    sc = apool.tile([128, 2, 2, HALF], f16, tag="sc")
                    nc.vector.tensor_tensor(
                        sc[:], sa[:, :, :, 0:HALF], sa[:, :, :, HALF:BAND],
                        op=MIN)
                    nc.vector.tensor_reduce(
                        outt[:, 4 * s:4 * s + 4], sc[:], axis=X, op=MIN)
                else:
                    # per-chunk matmuls (N=160) so one batched DVE
                    # tensor_reduce can drain all 4 chunks from PSUM fp32;
                    # row groups alternate for PE tile concurrency
                    ps = ppoold.tile([128, 4, 256], f32, tag="psd")
                    for cb in range(2):
                        for h in range(2):
                            wa, ra, p0, j = pack_aps(s, h)
                            nc.tensor.matmul(
                                ps[:, 2 * h + cb, 0:BAND],
                                wa, ra[:, cb, :],
                                start=True, stop=True, tile_position=(p0, 0),
                            )
                    nc.vector.tensor_reduce(
                        outt[:, 4 * s:4 * s + 4],
                        ps[:, :, 0:BAND], axis=X, op=MIN)
                if s == NSUPER // 2 - 1:
                    nc.sync.dma_start(out_d[:, 0:NBLK // 2],
                                      outt[:, 0:NBLK // 2])
            nc.sync.dma_start(out_d[:, NBLK // 2:], outt[:, NBLK // 2:])
    nc.compile()
    return nc


def _morton_key(pts):
    rng = pts.max(0) - pts.min(0)
    q = ((pts - pts.min(0)) / (rng + 1e-9) * 1023).astype(np.uint64)

    def spread(x):
        x = x & np.uint64(0x3FF)
        x = (x | (x << np.uint64(16))) & np.uint64(0x30000FF)
        x = (x | (x << np.uint64(8))) & np.uint64(0x300F00F)
        x = (x | (x << np.uint64(4))) & np.uint64(0x30C30C3)
        x = (x | (x << np.uint64(2))) & np.uint64(0x9249249)
        return x

    return (spread(q[:, 0]) | (spread(q[:, 1]) << np.uint64(1))
            | (spread(q[:, 2]) << np.uint64(2)))


def _prep_core(X, Y):
    """Host prep for one (batch, direction): returns in_map plus the metadata
    needed to verify and assemble the result."""
    xo = np.argsort(X[:, AXIS], kind="stable")
    yo = np.argsort(Y[:, AXIS], kind="stable")
    Xs = X[xo]
    Ys = Y[yo]
    X2 = (Xs.astype(np.float64) ** 2).sum(1)
    Y2 = (Ys.astype(np.float64) ** 2).sum(1)
    zx = Xs[:, AXIS].astype(np.float64)
    zy = Ys[:, AXIS].astype(np.float64)

    # gap to nearest excluded Y along the sort axis, per query
    i = np.arange(M)
    c = i // SUB
    lo_pos = SUB * c - GUARD         # first included Y position
    hi_pos = SUB * c + SUB + GUARD   # first excluded upper position
    gap = np.full(M, np.inf)
    has_lo = lo_pos > 0
    gap[has_lo] = zx[has_lo] - zy[lo_pos[has_lo] - 1]
    has_hi = hi_pos < P
    gap[has_hi] = np.minimum(gap[has_hi], zy[hi_pos[has_hi]] - zx[has_hi])
    gap = np.maximum(gap, 0.0)

    # conservative NN-distance upper bound via Morton-order neighbors
    allpts = np.concatenate([Xs, Ys]).astype(np.float64)
    mk = _morton_key(allpts)
    inv = np.empty(2 * M, dtype=np.int64)
    inv[np.argsort(mk, kind="stable")] = np.arange(2 * M)
    y_rank = inv[M:]
    order_y = np.argsort(y_rank, kind="stable")
    sorted_ranks = y_rank[order_y]
    K = 16
    idx = np.searchsorted(sorted_ranks, inv[:M])
    cand = np.clip(idx[:, None] + np.arange(-K, K)[None, :], 0, M - 1)
    cands = order_y[cand]
    d2 = ((Xs[:, None, :].astype(np.float64) - Ys[cands].astype(np.float64)) ** 2).sum(-1)
    d_cap2 = d2.min(1)

    hard = np.flatnonzero(~(d_cap2 <= (gap * gap) * 0.98))

    # fp16 hi/lo decomposition of SCALE*X and SCALE*Y; device computes
    # SCALE^2 * (|x|^2 - 2 x.y + |y|^2) in fp32 PSUM via K=13 rows:
    #   r0-2: -2*a_d * c_d     r3-5: -2*a_d * e_d     r6-8: -2*b_d * c_d
    #   r9:   1 * w_hi         r10:  1 * w_lo
    #   r11:  v_hi * 1         r12:  v_lo * 1
    # where a+b ~ SCALE*x, c+e ~ SCALE*y, w_hi+w_lo ~ |SCALE*y|^2,
    # v_hi+v_lo ~ |SCALE*x|^2.
    Xss = (SCALE * Xs).astype(np.float64)
    Yss = (SCALE * Ys).astype(np.float64)
    a = Xss.astype(np.float16)
    bb = (Xss - a.astype(np.float64)).astype(np.float16)
    cc = Yss.astype(np.float16)
    e = (Yss - cc.astype(np.float64)).astype(np.float16)
    w = (Yss ** 2).sum(1)
    wh = w.astype(np.float16)
    wl = (w - wh.astype(np.float64)).astype(np.float16)
    v = (Xss ** 2).sum(1)
    vh = v.astype(np.float16)
    vl = (v - vh.astype(np.float64)).astype(np.float16)

    na = (-2.0 * a.astype(np.float64)).astype(np.float16)  # exact: x2 of fp16
    nb = (-2.0 * bb.astype(np.float64)).astype(np.float16)

    wt = np.empty((KROWS, M), dtype=np.float16)
    wt[0:3, :] = na.T
    wt[3:6, :] = na.T
    wt[6:9, :] = nb.T
    wt[9:11, :] = 1.0
    wt[11, :] = vh
    wt[12, :] = vl

    rt = np.empty((KROWS, P + 2 * PAD), dtype=np.float16)
    ccT = cc.T
    eeT = e.T
    # edge-duplicate padding: repeats of the first/last sorted reference
    # point - real candidates, can never lower a min below the true min.
    for cols, sl in ((slice(0, PAD), 0), (slice(PAD + P, P + 2 * PAD), P - 1)):
        rt[0:3, cols] = ccT[:, sl:sl + 1]
        rt[3:6, cols] = eeT[:, sl:sl + 1]
        rt[6:9, cols] = ccT[:, sl:sl + 1]
        rt[9, cols] = wh[sl]
        rt[10, cols] = wl[sl]
    rt[0:3, PAD:PAD + P] = ccT
    rt[3:6, PAD:PAD + P] = eeT
    rt[6:9, PAD:PAD + P] = ccT
    rt[9, PAD:PAD + P] = wh
    rt[10, PAD:PAD + P] = wl
    rt[11:13, :] = 1.0

    # pack layout: pack p = chunks (2p, 2p+1) stacked along K (rows 0-12 and
    # 13-25) at PE row group 32*(p%4), local slot j=p//4. Moving data is
    # block-diagonal: block 0 carries chunk 2p's window on rows 0-12 (rows
    # 13-25 zero), block 1 carries chunk 2p+1's window on rows 13-25.
    wt_l = np.zeros((128, WTC), dtype=np.float16)
    rt_l = np.zeros((128, 2, NJ, BAND), dtype=np.float16)
    for p in range(NPACK):
        rg, j = p % 4, p // 4
        p0 = 32 * rg
        ca, cb = 2 * p, 2 * p + 1
        wt_l[p0:p0 + KROWS, SUB * j:SUB * (j + 1)] = wt[:, SUB * ca:SUB * (ca + 1)]
        wt_l[p0 + KROWS:p0 + K2, SUB * j:SUB * (j + 1)] = wt[:, SUB * cb:SUB * (cb + 1)]
        rt_l[p0:p0 + KROWS, 0, j, :] = rt[:, SUB * ca:SUB * ca + BAND]
        rt_l[p0 + KROWS:p0 + K2, 1, j, :] = rt[:, SUB * cb:SUB * cb + BAND]

    return {"wt": wt_l, "rt": rt_l}, {
        "Xs": Xs.astype(np.float64), "Ys": Ys.astype(np.float64),
        "X2": X2, "Y2": Y2, "gap": gap, "hard": hard,
    }


def _exact_rows(meta, idx):
    """Exact NN distance (float64) for query rows idx against all of Y."""
    Xb = meta["Xs"][idx]
    D = meta["X2"][idx][:, None] + meta["Y2"][None, :] - 2.0 * (Xb @ meta["Ys"].T)
    return D.min(axis=1)


def _post_core(out, meta):
    """Combine device output into sum over queries of min-D (float64)."""
    inv_s2 = 1.0 / (SCALE * SCALE)
    dmin = out.T.reshape(M).astype(np.float64) * inv_s2

    if len(meta["hard"]):
        dmin[meta["hard"]] = _exact_rows(meta, meta["hard"])

    # soundness check for window-only points: device numeric margin includes
    # the fp16 rounding of intermediates (rel ~8e-3) plus fp16 product noise
    g2 = meta["gap"] * meta["gap"]
    ok = dmin <= g2 - 2e-3 * inv_s2 - 8e-3 * np.abs(dmin)
    ok[meta["hard"]] = True
    bad = np.flatnonzero(~ok)
    if len(bad):
        dmin[bad] = _exact_rows(meta, bad)
    if os.environ.get("CHAMFER_DEBUG"):
        print(f"  host-recomputed: hard={len(meta['hard'])} bad={len(bad)}")
    return dmin.sum()


def _install_axon_profile_hook():
    """Make trace=True work under axon when the image's antenv lacks
    axon_hooks: inject a shim module wired to the ctypes NTFF driver."""
    import sys
    import types
    try:
        from antenv.axon_hooks import get_axon_ntff_profile_hook  # noqa: F401
        return
    except ImportError:
        pass
    try:
        import antenv
        from trn_agent_boot.trn_boot import _ntff_profile_via_ctypes
        hook = _ntff_profile_via_ctypes("/opt/axon/libaxon_pjrt.so")
    except Exception:
        hook = None
    mod = types.ModuleType("antenv.axon_hooks")
    state = {"h": hook}
    mod.get_axon_ntff_profile_hook = lambda: state["h"]
    mod.set_axon_ntff_profile_hook = lambda h: state.__setitem__("h", h)
    sys.modules["antenv.axon_hooks"] = mod
    try:
        antenv.axon_hooks = mod
    except Exception:
        pass


def kernel(x_hat, points, likelihoods):
    from concourse.bass_utils import run_bass_kernel_spmd
    global LAST_RESULTS

    trace = bool(int(os.environ.get("CHAMFER_TRACE", "0")))
    if trace:
        _install_axon_profile_hook()

    if "nc" not in _CACHE:
        _CACHE["nc"] = _build_bass()
    nc = _CACHE["nc"]

    in_maps, metas = [], []
    for core in range(8):
        b, d = core // 2, core % 2
        X = x_hat[b] if d == 0 else points[b]
        Y = points[b] if d == 0 else x_hat[b]
        m, meta = _prep_core(np.asarray(X), np.asarray(Y))
        in_maps.append(m)
        metas.append(meta)

    res = run_bass_kernel_spmd(
        nc, in_maps, core_ids=list(range(8)), trace=trace,
    )
    LAST_RESULTS = res

    sums = [_post_core(res.results[c]["out"], metas[c]) for c in range(8)]
    cham_x = sum(sums[c] for c in range(8) if c % 2 == 0) / (B * M)
    cham_y = sum(sums[c] for c in range(8) if c % 2 == 1) / (B * P)
    rec = cham_x + cham_y

    lik = np.asarray(likelihoods, dtype=np.float64)
    bpp = np.log2(lik).sum() / (-(B * P))

    loss = bpp + LMBDA * rec
    return np.array([loss, bpp, rec], dtype=np.float32)


# revision 9
# speedup vs baseline: 1.0509x; 1.0036x over previous
"""Chamfer rate-distortion loss on 8 TRN2 NeuronCores.

Layout: 8 cores = 4 batches x 2 chamfer directions. Each core computes, for
its (batch, direction), the per-point nearest-neighbor squared distance of
8192 query points X against 8192 reference points Y.

Device algorithm per core (v3):
  - X and Y are pre-sorted (host) along coordinate AXIS.
  - matmul trick (fp16 hi/lo split, K=13 rows/chunk, ~1e-6 rel precision):
    PSUM[m,p] = SCALE^2*(|x_m|^2 - 2 x_m.y_p + |y_p|^2) = SCALE^2*D[m,p] >= 0.
  - 64 chunks of 128 sorted queries each scan a 160-wide window of sorted Y
    (guard G=16 each side); edges padded with duplicates of the extreme
    real points (which can never lower a min below the true min).
  - K-packing: 2 chunks stacked along the contraction dim (26 rows) with
    block-diagonal moving data (the off-chunk halves are zero), so both
    chunks' matmuls share one stationary tile. The 32 packs rotate over PE
    row groups 0/32/64/96 (tile_position): consecutive matmuls hit
    different row groups, so the PE runs them concurrently (measured
    ~4-way overlap) and pulls weights ahead of in-flight matmuls.
  - PSUM super-tiles [128, 4chunks, 256] (2 banks); the min-reduction is
    split across two engines to overlap:
      * TR supers: one DVE tensor_reduce(min) straight from PSUM fp32.
      * SOFTMIN supers: one ACT exp-activation per chunk with per-query
        scale/bias ([128,1] APs) and fused sum accumulation:
        accum = sum_p exp(beta_m*(Vhat_m - V[m,p])); the host recovers
        min_p V = Vhat - ln(accum)/beta up to a small downward bias
        (~1e-4 rel on this data, two orders under the tolerance). Rows
        where the recovered value is non-finite or above the gap
        certificate are recomputed exactly on the host.

Exactness: for a query x, every Y outside its window differs from x along
the sort axis by at least gap(x), so any excluded point has D >= gap^2.
The host flags points whose Morton-candidate NN upper bound exceeds their
gap ("hard" points, data-dependent but sound) and recomputes them exactly
on the host; a post-hoc check dmin <= gap^2 - margin catches any residual
device noise and falls back to exact host recompute for those rows.
"""

import os

import numpy as np

B, M, P = 4, 8192, 8192
AXIS = 2
SUB = 128            # chunk: 128 sorted queries share one window
GUARD = 16           # guaranteed positions each side beyond the chunk span
BAND = SUB + 2 * GUARD   # 160 window width per chunk
PAD = GUARD          # edge-dup pad; window for chunk c = rt cols [128c, 128c+160)
NBLK = M // SUB      # 64 chunks
NPACK = NBLK // 2    # 32 two-chunk packs
NSUPER = NPACK // 2  # 16 PSUM super-tiles (4 chunks each)
NJ = NPACK // 4      # 8 local packs per row group
KROWS = 13           # fp16 hi/lo decomposition rows (see _prep_core)
K2 = 2 * KROWS       # 26 contraction rows per pack
WTC = NJ * SUB       # 1024 weight cols per row group line
SCALE = 32.0         # coordinate pre-scale; device min is SCALE^2 * real
LMBDA = 5.0
SOFT_C = 80.0        # softmin exponent budget
V_FLOOR = 0.1        # clamp for beta = SOFT_C / max(Vhat, V_FLOOR)
# consumer per super-tile: s = ACT softmin (4 exp ops), r = DVE tensor_reduce
PATHS = "rsrrsrrsrrsrrsrr"

_CACHE = {}
LAST_RESULTS = None


def _build_bass():
    import concourse.tile as tile
    from concourse import bacc, mybir

    nc = bacc.Bacc(None, target_bir_lowering=False, debug=False)
    f32 = mybir.dt.float32
    f16 = mybir.dt.float16
    bf16 = mybir.dt.bfloat16
    MIN = mybir.AluOpType.min
    X = mybir.AxisListType.X
    EXP = mybir.ActivationFunctionType.Exp

    wt_d = nc.dram_tensor("wt", [128, WTC], f16, kind="ExternalInput")
    rt_d = nc.dram_tensor("rt", [128, 2, NJ, BAND], f16, kind="ExternalInput")
    sb_d = nc.dram_tensor("sb", [128, 2, NBLK], f32, kind="ExternalInput")
    out_d = nc.dram_tensor("out", [128, NBLK], f32, kind="ExternalOutput")

    with tile.TileContext(nc) as tc:
        with (
            tc.tile_pool(name="const", bufs=1) as cpool,
            tc.tile_pool(name="outp", bufs=1) as opool,
            tc.tile_pool(name="sba", bufs=2) as apool,
            tc.tile_pool(name="psum", bufs=4, space="PSUM") as ppool,
        ):
            wtile = cpool.tile([128, WTC], f16)
            rtile = cpool.tile([128, 2, NJ, BAND], f16)
            sbt = cpool.tile([128, 2, NBLK], f32)
            outt = opool.tile([128, NBLK], f32)

            # input DMA: minimal first slices so the first matmuls start
            # early; the bulk streams behind on both queues
            nc.sync.dma_start(wtile[:, 0:SUB], wt_d[:, 0:SUB])
            nc.sync.dma_start(rtile[:, 0, 0:1, :], rt_d[:, 0, 0:1, :])
            nc.sync.dma_start(rtile[:, 1, 0:1, :], rt_d[:, 1, 0:1, :])
            nc.scalar.dma_start(sbt[:], sb_d[:])
            nc.scalar.dma_start(wtile[:, SUB:WTC], wt_d[:, SUB:WTC])
            nc.sync.dma_start(rtile[:, 0, 1:NJ, :], rt_d[:, 0, 1:NJ, :])
            nc.scalar.dma_start(rtile[:, 1, 1:NJ, :], rt_d[:, 1, 1:NJ, :])

            for s in range(NSUPER):
                ps = ppool.tile([128, 4, 256], f32, tag="ps")
                for cb in range(2):
                    for h in range(2):
                        p = 2 * s + h          # global pack index
                        rg = p % 4             # PE row group (rotates)
                        j = p // 4             # local pack within row group
                        p0 = 32 * rg
                        nc.tensor.matmul(
                            ps[:, 2 * h + cb, 0:BAND],
                            wtile[p0:p0 + K2, SUB * j:SUB * (j + 1)],
                            rtile[p0:p0 + K2, cb, j, :],
                            start=True, stop=True,
                            tile_position=(p0, 0),
                        )
                if PATHS[s] == "r":
                    # one DVE reduce drains all 4 chunks from PSUM fp32
                    nc.vector.tensor_reduce(
                        outt[:, 4 * s:4 * s + 4],
                        ps[:, :, 0:BAND], axis=X, op=MIN)
                else:
                    # ACT softmin: accum = sum_p exp(beta*(Vhat - V_p));
                    # scale/bias are per-query [128,1] columns
                    for c in range(4):
                        col = 4 * s + c
                        sc = apool.tile([128, BAND], f32, tag="sc")
                        nc.scalar.activation(
                            sc[:], ps[:, c, 0:BAND], EXP,
                            bias=sbt[:, 1, col:col + 1],
                            scale=sbt[:, 0, col:col + 1],
                            accum_out=outt[:, col:col + 1],
                        )
                if s == NSUPER // 2 - 1:
                    nc.sync.dma_start(out_d[:, 0:NBLK // 2],
                                      outt[:, 0:NBLK // 2])
            nc.sync.dma_start(out_d[:, NBLK // 2:], outt[:, NBLK // 2:])
    nc.compile()
    return nc


def _morton_key(pts):
    rng = pts.max(0) - pts.min(0)
    q = ((pts - pts.min(0)) / (rng + 1e-9) * 1023).astype(np.uint64)

    def spread(x):
        x = x & np.uint64(0x3FF)
        x = (x | (x << np.uint64(16))) & np.uint64(0x30000FF)
        x = (x | (x << np.uint64(8))) & np.uint64(0x300F00F)
        x = (x | (x << np.uint64(4))) & np.uint64(0x30C30C3)
        x = (x | (x << np.uint64(2))) & np.uint64(0x9249249)
        return x

    return (spread(q[:, 0]) | (spread(q[:, 1]) << np.uint64(1))
            | (spread(q[:, 2]) << np.uint64(2)))


def _prep_core(X, Y):
    """Host prep for one (batch, direction): returns in_map plus the metadata
    needed to verify and assemble the result."""
    xo = np.argsort(X[:, AXIS], kind="stable")
    yo = np.argsort(Y[:, AXIS], kind="stable")
    Xs = X[xo]
    Ys = Y[yo]
    X2 = (Xs.astype(np.float64) ** 2).sum(1)
    Y2 = (Ys.astype(np.float64) ** 2).sum(1)
    zx = Xs[:, AXIS].astype(np.float64)
    zy = Ys[:, AXIS].astype(np.float64)

    # gap to nearest excluded Y along the sort axis, per query
    i = np.arange(M)
    c = i // SUB
    lo_pos = SUB * c - GUARD         # first included Y position
    hi_pos = SUB * c + SUB + GUARD   # first excluded upper position
    gap = np.full(M, np.inf)
    has_lo = lo_pos > 0
    gap[has_lo] = zx[has_lo] - zy[lo_pos[has_lo] - 1]
    has_hi = hi_pos < P
    gap[has_hi] = np.minimum(gap[has_hi], zy[hi_pos[has_hi]] - zx[has_hi])
    gap = np.maximum(gap, 0.0)

    # conservative NN-distance upper bound via Morton-order neighbors
    allpts = np.concatenate([Xs, Ys]).astype(np.float64)
    mk = _morton_key(allpts)
    inv = np.empty(2 * M, dtype=np.int64)
    inv[np.argsort(mk, kind="stable")] = np.arange(2 * M)
    y_rank = inv[M:]
    order_y = np.argsort(y_rank, kind="stable")
    sorted_ranks = y_rank[order_y]
    K = 16
    idx = np.searchsorted(sorted_ranks, inv[:M])
    cand = np.clip(idx[:, None] + np.arange(-K, K)[None, :], 0, M - 1)
    cands = order_y[cand]
    d2 = ((Xs[:, None, :].astype(np.float64) - Ys[cands].astype(np.float64)) ** 2).sum(-1)
    d_cap2 = d2.min(1)

    hard = np.flatnonzero(~(d_cap2 <= (gap * gap) * 0.98))

    # softmin per-query scale/bias: beta = C/max(Vhat, floor), Vhat = S^2*cap
    Vhat = (SCALE * SCALE) * d_cap2
    beta = SOFT_C / np.maximum(Vhat, V_FLOOR)
    # sb layout: [128, 2, NBLK]: row m, chunk c -> query 128c+m
    sb = np.empty((128, 2, NBLK), dtype=np.float32)
    sb[:, 0, :] = (-beta).reshape(NBLK, SUB).T
    sb[:, 1, :] = (beta * Vhat).reshape(NBLK, SUB).T

    # fp16 hi/lo decomposition of SCALE*X and SCALE*Y; device computes
    # SCALE^2 * (|x|^2 - 2 x.y + |y|^2) in fp32 PSUM via K=13 rows:
    #   r0-2: -2*a_d * c_d     r3-5: -2*a_d * e_d     r6-8: -2*b_d * c_d
    #   r9:   1 * w_hi         r10:  1 * w_lo
    #   r11:  v_hi * 1         r12:  v_lo * 1
    # where a+b ~ SCALE*x, c+e ~ SCALE*y, w_hi+w_lo ~ |SCALE*y|^2,
    # v_hi+v_lo ~ |SCALE*x|^2.
    Xss = (SCALE * Xs).astype(np.float64)
    Yss = (SCALE * Ys).astype(np.float64)
    a = Xss.astype(np.float16)
    bb = (Xss - a.astype(np.float64)).astype(np.float16)
    cc = Yss.astype(np.float16)
    e = (Yss - cc.astype(np.float64)).astype(np.float16)
    w = (Yss ** 2).sum(1)
    wh = w.astype(np.float16)
    wl = (w - wh.astype(np.float64)).astype(np.float16)
    v = (Xss ** 2).sum(1)
    vh = v.astype(np.float16)
    vl = (v - vh.astype(np.float64)).astype(np.float16)

    na = (-2.0 * a.astype(np.float64)).astype(np.float16)  # exact: x2 of fp16
    nb = (-2.0 * bb.astype(np.float64)).astype(np.float16)

    wt = np.empty((KROWS, M), dtype=np.float16)
    wt[0:3, :] = na.T
    wt[3:6, :] = na.T
    wt[6:9, :] = nb.T
    wt[9:11, :] = 1.0
    wt[11, :] = vh
    wt[12, :] = vl

    rt = np.empty((KROWS, P + 2 * PAD), dtype=np.float16)
    ccT = cc.T
    eeT = e.T
    # edge-duplicate padding: repeats of the first/last sorted reference
    # point - real candidates, can never lower a min below the true min.
    for cols, sl in ((slice(0, PAD), 0), (slice(PAD + P, P + 2 * PAD), P - 1)):
        rt[0:3, cols] = ccT[:, sl:sl + 1]
        rt[3:6, cols] = eeT[:, sl:sl + 1]
        rt[6:9, cols] = ccT[:, sl:sl + 1]
        rt[9, cols] = wh[sl]
        rt[10, cols] = wl[sl]
    rt[0:3, PAD:PAD + P] = ccT
    rt[3:6, PAD:PAD + P] = eeT
    rt[6:9, PAD:PAD + P] = ccT
    rt[9, PAD:PAD + P] = wh
    rt[10, PAD:PAD + P] = wl
    rt[11:13, :] = 1.0

    # pack layout: pack p = chunks (2p, 2p+1) stacked along K (rows 0-12 and
    # 13-25) at PE row group 32*(p%4), local slot j=p//4. Moving data is
    # block-diagonal: block 0 carries chunk 2p's window on rows 0-12 (rows
    # 13-25 zero), block 1 carries chunk 2p+1's window on rows 13-25.
    wt_l = np.zeros((128, WTC), dtype=np.float16)
    rt_l = np.zeros((128, 2, NJ, BAND), dtype=np.float16)
    for p in range(NPACK):
        rg, j = p % 4, p // 4
        p0 = 32 * rg
        ca, cb = 2 * p, 2 * p + 1
        wt_l[p0:p0 + KROWS, SUB * j:SUB * (j + 1)] = wt[:, SUB * ca:SUB * (ca + 1)]
        wt_l[p0 + KROWS:p0 + K2, SUB * j:SUB * (j + 1)] = wt[:, SUB * cb:SUB * (cb + 1)]
        rt_l[p0:p0 + KROWS, 0, j, :] = rt[:, SUB * ca:SUB * ca + BAND]
        rt_l[p0 + KROWS:p0 + K2, 1, j, :] = rt[:, SUB * cb:SUB * cb + BAND]

    return {"wt": wt_l, "rt": rt_l, "sb": sb}, {
        "Xs": Xs.astype(np.float64), "Ys": Ys.astype(np.float64),
        "X2": X2, "Y2": Y2, "gap": gap, "hard": hard,
        "Vhat": Vhat, "beta": beta,
    }


def _exact_rows(meta, idx):
    """Exact NN distance (float64) for query rows idx against all of Y."""
    Xb = meta["Xs"][idx]
    D = meta["X2"][idx][:, None] + meta["Y2"][None, :] - 2.0 * (Xb @ meta["Ys"].T)
    return D.min(axis=1)


def _raw_dmin(out, meta):
    """Device output -> per-query min-D estimate (float64), before the
    hard/bad host recomputes."""
    inv_s2 = 1.0 / (SCALE * SCALE)
    vals = out.T.astype(np.float64).copy()      # [NBLK, 128]
    with np.errstate(divide="ignore", invalid="ignore"):
        for s in range(NSUPER):
            if PATHS[s] == "s":
                cs = slice(4 * s, 4 * s + 4)
                q = np.arange(4 * s * SUB, (4 * s + 4) * SUB)
                vh = meta["Vhat"][q].reshape(4, SUB)
                be = meta["beta"][q].reshape(4, SUB)
                vals[cs] = vh - np.log(vals[cs]) / be
    return vals.reshape(M) * inv_s2


def _post_core(out, meta):
    """Combine device output into sum over queries of min-D (float64)."""
    inv_s2 = 1.0 / (SCALE * SCALE)
    dmin = _raw_dmin(out, meta)

    if len(meta["hard"]):
        dmin[meta["hard"]] = _exact_rows(meta, meta["hard"])

    # soundness check for window-only points: device numeric margin covers
    # matmul fp16 product noise and the softmin recovery; non-finite or
    # negative values fall back to exact as well
    g2 = meta["gap"] * meta["gap"]
    ok = (dmin <= g2 - 2e-3 * inv_s2 - 8e-3 * np.abs(dmin))
    ok &= np.isfinite(dmin) & (dmin > -1e-3)
    ok[meta["hard"]] = True
    bad = np.flatnonzero(~ok)
    if len(bad):
        dmin[bad] = _exact_rows(meta, bad)
    if os.environ.get("CHAMFER_DEBUG"):
        print(f"  host-recomputed: hard={len(meta['hard'])} bad={len(bad)}")
    return dmin.sum()


def _install_axon_profile_hook():
    """Make trace=True work under axon when the image's antenv lacks
    axon_hooks: inject a shim module wired to the ctypes NTFF driver."""
    import sys
    import types
    try:
        from antenv.axon_hooks import get_axon_ntff_profile_hook  # noqa: F401
        return
    except ImportError:
        pass
    try:
        import antenv
        from trn_agent_boot.trn_boot import _ntff_profile_via_ctypes
        hook = _ntff_profile_via_ctypes("/opt/axon/libaxon_pjrt.so")
    except Exception:
        hook = None
    mod = types.ModuleType("antenv.axon_hooks")
    state = {"h": hook}
    mod.get_axon_ntff_profile_hook = lambda: state["h"]
    mod.set_axon_ntff_profile_hook = lambda h: state.__setitem__("h", h)
    sys.modules["antenv.axon_hooks"] = mod
    try:
        antenv.axon_hooks = mod
    except Exception:
        pass


def kernel(x_hat, points, likelihoods):
    from concourse.bass_utils import run_bass_kernel_spmd
    global LAST_RESULTS

    trace = bool(int(os.environ.get("CHAMFER_TRACE", "0")))
    if trace:
        _install_axon_profile_hook()

    if "nc" not in _CACHE:
        _CACHE["nc"] = _build_bass()
    nc = _CACHE["nc"]

    in_maps, metas = [], []
    for core in range(8):
        b, d = core // 2, core % 2
        X = x_hat[b] if d == 0 else points[b]
        Y = points[b] if d == 0 else x_hat[b]
        m, meta = _prep_core(np.asarray(X), np.asarray(Y))
        in_maps.append(m)
        metas.append(meta)

    res = run_bass_kernel_spmd(
        nc, in_maps, core_ids=list(range(8)), trace=trace,
    )
    LAST_RESULTS = res

    sums = [_post_core(res.results[c]["out"], metas[c]) for c in range(8)]
    cham_x = sum(sums[c] for c in range(8) if c % 2 == 0) / (B * M)
    cham_y = sum(sums[c] for c in range(8) if c % 2 == 1) / (B * P)
    rec = cham_x + cham_y

    lik = np.asarray(likelihoods, dtype=np.float64)
    bpp = np.log2(lik).sum() / (-(B * P))

    loss = bpp + LMBDA * rec
    return np.array([loss, bpp, rec], dtype=np.float32)


# revision 12
# speedup vs baseline: 1.2363x; 1.1764x over previous
"""Chamfer rate-distortion loss on 8 TRN2 NeuronCores.

Layout: 8 cores = 4 batches x 2 chamfer directions. Each core computes, for
its (batch, direction), the per-point nearest-neighbor squared distance of
8192 query points X against 8192 reference points Y.

Device algorithm per core (v4):
  - X and Y are pre-sorted (host) along coordinate AXIS.
  - matmul trick (fp16 hi/lo split, K=13 rows/chunk, ~1e-6 rel precision):
    PSUM[m,p] = SCALE^2*(|x_m|^2 - 2 x_m.y_p + |y_p|^2) = SCALE^2*D[m,p] >= 0.
  - 64 chunks of 128 sorted queries each scan a BAND=128-wide window of
    sorted Y whose start is chosen per chunk (host) to cover each query's
    z-ball of radius sqrt(cap), where cap is a Morton-candidate NN upper
    bound. Covered queries are provably exact (any excluded point differs
    in z by >= sqrt(cap) so its D >= cap >= true min, and the true NN lies
    inside the ball hence the window); uncovered queries are recomputed
    exactly on the host.
  - K-packing: 2 chunks stacked along the contraction dim (26 rows) with
    block-diagonal moving data (the off-chunk halves are zero), so both
    chunks' matmuls share one stationary tile. The 32 packs rotate over PE
    row groups 0/32/64/96 (tile_position): consecutive matmuls hit
    different row groups, so the PE runs them concurrently (~4-way) and
    pulls weights ahead of in-flight matmuls.
  - PSUM super-tiles [128, 8chunks, 128] (2 banks, tightly packed); the
    min-reduction is split across two engines to overlap:
      * head chunks: one DVE tensor_reduce(min) straight from PSUM fp32.
      * tail chunks: ACT softmin - one exp-activation per chunk with
        per-query scale/bias ([128,1] APs) and fused sum accumulation:
        accum = sum_p exp(beta_m*(Vhat_m - V[m,p])); the host recovers
        min_p V = Vhat - ln(accum)/beta up to a small downward bias
        (~1e-4 rel on this data, two orders under the tolerance). Rows
        where the recovered value is non-finite or above cap + margin are
        recomputed exactly on the host.
"""

import os

import numpy as np

B, M, P = 4, 8192, 8192
AXIS = 2
SUB = 128            # chunk: 128 sorted queries share one window
BAND = 128           # window width per chunk (data-driven start)
NBLK = M // SUB      # 64 chunks
NPACK = NBLK // 2    # 32 two-chunk packs
NSUPER = 8           # PSUM super-tiles (8 chunks each)
NJ = NPACK // 4      # 8 local packs per row group
KROWS = 13           # fp16 hi/lo decomposition rows (see _prep_core)
K2 = 2 * KROWS       # 26 contraction rows per pack
WTC = NJ * SUB       # 1024 weight cols per row group line
SCALE = 32.0         # coordinate pre-scale; device min is SCALE^2 * real
LMBDA = 5.0
SOFT_C = 80.0        # softmin exponent budget
V_FLOOR = 0.1        # clamp for beta = SOFT_C / max(Vhat, V_FLOOR)
# softmin tail chunks per super-tile (rest reduce via DVE tensor_reduce)
TAILS = (0, 0, 0, 0, 0, 0, 0, 0)

_CACHE = {}
LAST_RESULTS = None


def _soft_chunks():
    out = []
    for s in range(NSUPER):
        for c in range(8 - TAILS[s], 8):
            out.append(8 * s + c)
    return out


SOFT_SET = frozenset(_soft_chunks())


def _build_bass():
    import concourse.tile as tile
    from concourse import bacc, mybir

    nc = bacc.Bacc(None, target_bir_lowering=False, debug=False)
    f32 = mybir.dt.float32
    f16 = mybir.dt.float16
    MIN = mybir.AluOpType.min
    X = mybir.AxisListType.X
    EXP = mybir.ActivationFunctionType.Exp

    # first block: weights j=0 (cols 0:128) + windows of packs 0-3
    # (blk0 j0 at cols 128:256, blk1 j0 at 256:384) - one early DMA
    ft_d = nc.dram_tensor("ft", [128, 3 * SUB], f16, kind="ExternalInput")
    wt_d = nc.dram_tensor("wt", [128, WTC - SUB], f16, kind="ExternalInput")
    rt_d = nc.dram_tensor("rt", [128, 2, NJ - 1, BAND], f16, kind="ExternalInput")
    sb_d = nc.dram_tensor("sb", [128, 2, NBLK], f32, kind="ExternalInput")
    out_d = nc.dram_tensor("out", [128, NBLK], f32, kind="ExternalOutput")

    with tile.TileContext(nc) as tc:
        with (
            tc.tile_pool(name="const", bufs=1) as cpool,
            tc.tile_pool(name="outp", bufs=1) as opool,
            tc.tile_pool(name="sba", bufs=2) as apool,
            tc.tile_pool(name="psum", bufs=2, space="PSUM") as ppool,
        ):
            ftile = cpool.tile([128, 3 * SUB], f16)
            wtile = cpool.tile([128, WTC - SUB], f16)
            rtile = cpool.tile([128, 2, NJ - 1, BAND], f16)
            sbt = cpool.tile([128, 2, NBLK], f32)
            outt = opool.tile([128, NBLK], f32)

            nc.sync.dma_start(ftile[:], ft_d[:])
            nc.scalar.dma_start(wtile[:], wt_d[:])
            nc.sync.dma_start(rtile[:, 0, :, :], rt_d[:, 0, :, :])
            nc.scalar.dma_start(rtile[:, 1, :, :], rt_d[:, 1, :, :])
            nc.scalar.dma_start(sbt[:], sb_d[:])

            def mm_aps(p, cb):
                rg, j = p % 4, p // 4
                p0 = 32 * rg
                if j == 0:
                    wa = ftile[p0:p0 + K2, 0:SUB]
                    ra = ftile[p0:p0 + K2, SUB * (1 + cb):SUB * (2 + cb)]
                else:
                    wa = wtile[p0:p0 + K2, SUB * (j - 1):SUB * j]
                    ra = rtile[p0:p0 + K2, cb, j - 1, :]
                return wa, ra, p0

            for s in range(NSUPER):
                ps = ppool.tile([128, 8, 256], f32, tag="ps")
                for cb in range(2):
                    for h in range(4):
                        p = 4 * s + h          # pack; row group rotates
                        wa, ra, p0 = mm_aps(p, cb)
                        nc.tensor.matmul(
                            ps[:, 2 * h + cb, 0:BAND], wa, ra,
                            start=True, stop=True, tile_position=(p0, 0),
                        )
                nh = 8 - TAILS[s]
                # head chunks: one DVE reduce straight from PSUM fp32
                nc.vector.tensor_reduce(
                    outt[:, 8 * s:8 * s + nh],
                    ps[:, 0:nh, 0:BAND], axis=X, op=MIN)
                # tail chunks: ACT softmin with fused sum accumulation
                for c in range(nh, 8):
                    col = 8 * s + c
                    sc = apool.tile([128, BAND], f32, tag="sc")
                    nc.scalar.activation(
                        sc[:], ps[:, c, 0:BAND], EXP,
                        bias=sbt[:, 1, col:col + 1],
                        scale=sbt[:, 0, col:col + 1],
                        accum_out=outt[:, col:col + 1],
                    )
                if s == NSUPER // 2 - 1:
                    nc.sync.dma_start(out_d[:, 0:NBLK // 2],
                                      outt[:, 0:NBLK // 2])
            nc.sync.dma_start(out_d[:, NBLK // 2:], outt[:, NBLK // 2:])
    nc.compile()
    return nc


def _morton_key(pts):
    rng = pts.max(0) - pts.min(0)
    q = ((pts - pts.min(0)) / (rng + 1e-9) * 1023).astype(np.uint64)

    def spread(x):
        x = x & np.uint64(0x3FF)
        x = (x | (x << np.uint64(16))) & np.uint64(0x30000FF)
        x = (x | (x << np.uint64(8))) & np.uint64(0x300F00F)
        x = (x | (x << np.uint64(4))) & np.uint64(0x30C30C3)
        x = (x | (x << np.uint64(2))) & np.uint64(0x9249249)
        return x

    return (spread(q[:, 0]) | (spread(q[:, 1]) << np.uint64(1))
            | (spread(q[:, 2]) << np.uint64(2)))


def _prep_core(X, Y):
    """Host prep for one (batch, direction): returns in_map plus the metadata
    needed to verify and assemble the result."""
    xo = np.argsort(X[:, AXIS], kind="stable")
    yo = np.argsort(Y[:, AXIS], kind="stable")
    Xs = X[xo]
    Ys = Y[yo]
    X2 = (Xs.astype(np.float64) ** 2).sum(1)
    Y2 = (Ys.astype(np.float64) ** 2).sum(1)
    zx = Xs[:, AXIS].astype(np.float64)
    zy = Ys[:, AXIS].astype(np.float64)

    # NN-distance upper bound: Morton-order neighbors + z-sort neighbors
    allpts = np.concatenate([Xs, Ys]).astype(np.float64)
    mk = _morton_key(allpts)
    inv = np.empty(2 * M, dtype=np.int64)
    inv[np.argsort(mk, kind="stable")] = np.arange(2 * M)
    y_rank = inv[M:]
    order_y = np.argsort(y_rank, kind="stable")
    sorted_ranks = y_rank[order_y]
    K = 16
    idx = np.searchsorted(sorted_ranks, inv[:M])
    cand = np.clip(idx[:, None] + np.arange(-K, K)[None, :], 0, M - 1)
    cands = order_y[cand]
    zpos = np.searchsorted(zy, zx)
    zcand = np.clip(zpos[:, None] + np.arange(-8, 8)[None, :], 0, P - 1)
    cands = np.concatenate([cands, zcand], axis=1)
    d2 = ((Xs[:, None, :].astype(np.float64) - Ys[cands].astype(np.float64)) ** 2).sum(-1)
    d_cap2 = d2.min(1)

    # data-driven window starts: cover each query's z-ball [zx-r, zx+r];
    # the start maximizing coverage wins, uncovered rows go to the host
    r = np.sqrt(d_cap2)
    L = np.searchsorted(zy, zx - r, side="left")
    H = np.searchsorted(zy, zx + r, side="right")
    starts = np.empty(NBLK, dtype=np.int64)
    covered = np.zeros(M, dtype=bool)
    for c in range(NBLK):
        Q = slice(SUB * c, SUB * (c + 1))
        Lq, Hq = L[Q], H[Q]
        cs = np.unique(np.clip(np.concatenate([Hq - BAND, Lq]), 0, P - BAND))
        cov = (Lq[None, :] >= cs[:, None]) & (Hq[None, :] <= cs[:, None] + BAND)
        k = cov.sum(1).argmax()
        starts[c] = cs[k]
        covered[Q] = cov[k]
    hard = np.flatnonzero(~covered)

    # softmin per-query scale/bias: beta = C/max(Vhat, floor), Vhat = S^2*cap
    Vhat = (SCALE * SCALE) * d_cap2
    beta = SOFT_C / np.maximum(Vhat, V_FLOOR)
    sb = np.empty((128, 2, NBLK), dtype=np.float32)
    sb[:, 0, :] = (-beta).reshape(NBLK, SUB).T
    sb[:, 1, :] = (beta * Vhat).reshape(NBLK, SUB).T

    # fp16 hi/lo decomposition of SCALE*X and SCALE*Y; device computes
    # SCALE^2 * (|x|^2 - 2 x.y + |y|^2) in fp32 PSUM via K=13 rows:
    #   r0-2: -2*a_d * c_d     r3-5: -2*a_d * e_d     r6-8: -2*b_d * c_d
    #   r9:   1 * w_hi         r10:  1 * w_lo
    #   r11:  v_hi * 1         r12:  v_lo * 1
    # where a+b ~ SCALE*x, c+e ~ SCALE*y, w_hi+w_lo ~ |SCALE*y|^2,
    # v_hi+v_lo ~ |SCALE*x|^2.
    Xss = (SCALE * Xs).astype(np.float64)
    Yss = (SCALE * Ys).astype(np.float64)
    a = Xss.astype(np.float16)
    bb = (Xss - a.astype(np.float64)).astype(np.float16)
    cc = Yss.astype(np.float16)
    e = (Yss - cc.astype(np.float64)).astype(np.float16)
    w = (Yss ** 2).sum(1)
    wh = w.astype(np.float16)
    wl = (w - wh.astype(np.float64)).astype(np.float16)
    v = (Xss ** 2).sum(1)
    vh = v.astype(np.float16)
    vl = (v - vh.astype(np.float64)).astype(np.float16)

    na = (-2.0 * a.astype(np.float64)).astype(np.float16)  # exact: x2 of fp16
    nb = (-2.0 * bb.astype(np.float64)).astype(np.float16)

    wt = np.empty((KROWS, M), dtype=np.float16)
    wt[0:3, :] = na.T
    wt[3:6, :] = na.T
    wt[6:9, :] = nb.T
    wt[9:11, :] = 1.0
    wt[11, :] = vh
    wt[12, :] = vl

    rt = np.empty((KROWS, P), dtype=np.float16)
    rt[0:3, :] = cc.T
    rt[3:6, :] = e.T
    rt[6:9, :] = cc.T
    rt[9, :] = wh
    rt[10, :] = wl
    rt[11:13, :] = 1.0

    # pack layout: pack p = chunks (2p, 2p+1) stacked along K (rows 0-12 and
    # 13-25) at PE row group 32*(p%4), local slot j=p//4. Moving data is
    # block-diagonal: block 0 carries chunk 2p's window on rows 0-12 (rows
    # 13-25 zero), block 1 carries chunk 2p+1's window on rows 13-25.
    ft_l = np.zeros((128, 3 * SUB), dtype=np.float16)
    wt_l = np.zeros((128, WTC - SUB), dtype=np.float16)
    rt_l = np.zeros((128, 2, NJ - 1, BAND), dtype=np.float16)
    for p in range(NPACK):
        rg, j = p % 4, p // 4
        p0 = 32 * rg
        ca, cb = 2 * p, 2 * p + 1
        wblk_a = wt[:, SUB * ca:SUB * (ca + 1)]
        wblk_b = wt[:, SUB * cb:SUB * (cb + 1)]
        wina = rt[:, starts[ca]:starts[ca] + BAND]
        winb = rt[:, starts[cb]:starts[cb] + BAND]
        if j == 0:
            ft_l[p0:p0 + KROWS, 0:SUB] = wblk_a
            ft_l[p0 + KROWS:p0 + K2, 0:SUB] = wblk_b
            ft_l[p0:p0 + KROWS, SUB:2 * SUB] = wina
            ft_l[p0 + KROWS:p0 + K2, 2 * SUB:3 * SUB] = winb
        else:
            wt_l[p0:p0 + KROWS, SUB * (j - 1):SUB * j] = wblk_a
            wt_l[p0 + KROWS:p0 + K2, SUB * (j - 1):SUB * j] = wblk_b
            rt_l[p0:p0 + KROWS, 0, j - 1, :] = wina
            rt_l[p0 + KROWS:p0 + K2, 1, j - 1, :] = winb

    return {"ft": ft_l, "wt": wt_l, "rt": rt_l, "sb": sb}, {
        "Xs": Xs.astype(np.float64), "Ys": Ys.astype(np.float64),
        "X2": X2, "Y2": Y2, "cap2": d_cap2, "hard": hard,
        "Vhat": Vhat, "beta": beta, "starts": starts,
    }


def _exact_rows(meta, idx):
    """Exact NN distance (float64) for query rows idx against all of Y."""
    Xb = meta["Xs"][idx]
    D = meta["X2"][idx][:, None] + meta["Y2"][None, :] - 2.0 * (Xb @ meta["Ys"].T)
    return D.min(axis=1)


def _raw_dmin(out, meta):
    """Device output -> per-query min-D estimate (float64), before the
    hard/bad host recomputes."""
    inv_s2 = 1.0 / (SCALE * SCALE)
    vals = out.T.astype(np.float64).copy()      # [NBLK, 128]
    with np.errstate(divide="ignore", invalid="ignore", over="ignore"):
        for c in SOFT_SET:
            q = np.arange(c * SUB, (c + 1) * SUB)
            vals[c] = meta["Vhat"][q] - np.log(vals[c]) / meta["beta"][q]
    return vals.reshape(M) * inv_s2


def _post_core(out, meta):
    """Combine device output into sum over queries of min-D (float64)."""
    inv_s2 = 1.0 / (SCALE * SCALE)
    dmin = _raw_dmin(out, meta)

    if len(meta["hard"]):
        dmin[meta["hard"]] = _exact_rows(meta, meta["hard"])

    # soundness: covered rows must satisfy dmin <= cap (up to device noise
    # and softmin recovery margin); non-finite or negative fall back too
    ok = dmin <= meta["cap2"] + 2e-3 * inv_s2 + 8e-3 * np.abs(dmin)
    ok &= np.isfinite(dmin) & (dmin > -1e-3)
    ok[meta["hard"]] = True
    bad = np.flatnonzero(~ok)
    if len(bad):
        dmin[bad] = _exact_rows(meta, bad)
    if os.environ.get("CHAMFER_DEBUG"):
        print(f"  host-recomputed: hard={len(meta['hard'])} bad={len(bad)}")
    return dmin.sum()


def _install_axon_profile_hook():
    """Make trace=True work under axon when the image's antenv lacks
    axon_hooks: inject a shim module wired to the ctypes NTFF driver."""
    import sys
    import types
    try:
        from antenv.axon_hooks import get_axon_ntff_profile_hook  # noqa: F401
        return
    except ImportError:
        pass
    try:
        import antenv
        from trn_agent_boot.trn_boot import _ntff_profile_via_ctypes
        hook = _ntff_profile_via_ctypes("/opt/axon/libaxon_pjrt.so")
    except Exception:
        hook = None
    mod = types.ModuleType("antenv.axon_hooks")
    state = {"h": hook}
    mod.get_axon_ntff_profile_hook = lambda: state["h"]
    mod.set_axon_ntff_profile_hook = lambda h: state.__setitem__("h", h)
    sys.modules["antenv.axon_hooks"] = mod
    try:
        antenv.axon_hooks = mod
    except Exception:
        pass


def kernel(x_hat, points, likelihoods):
    from concourse.bass_utils import run_bass_kernel_spmd
    global LAST_RESULTS

    trace = bool(int(os.environ.get("CHAMFER_TRACE", "0")))
    if trace:
        _install_axon_profile_hook()

    if "nc" not in _CACHE:
        _CACHE["nc"] = _build_bass()
    nc = _CACHE["nc"]

    in_maps, metas = [], []
    for core in range(8):
        b, d = core // 2, core % 2
        X = x_hat[b] if d == 0 else points[b]
        Y = points[b] if d == 0 else x_hat[b]
        m, meta = _prep_core(np.asarray(X), np.asarray(Y))
        in_maps.append(m)
        metas.append(meta)

    res = run_bass_kernel_spmd(
        nc, in_maps, core_ids=list(range(8)), trace=trace,
    )
    LAST_RESULTS = res

    sums = [_post_core(res.results[c]["out"], metas[c]) for c in range(8)]
    cham_x = sum(sums[c] for c in range(8) if c % 2 == 0) / (B * M)
    cham_y = sum(sums[c] for c in range(8) if c % 2 == 1) / (B * P)
    rec = cham_x + cham_y

    lik = np.asarray(likelihoods, dtype=np.float64)
    bpp = np.log2(lik).sum() / (-(B * P))

    loss = bpp + LMBDA * rec
    return np.array([loss, bpp, rec], dtype=np.float32)
